# revision 4
# baseline (speedup 1.0000x reference)
"""Trainium2 Bass kernel for DerivativeNet.forward(u, direction='x').

out = eroded * (u[x+1]-u[x-1])/(2h) + edge1 * (u[x+1]-u[x])/h + edge2 * (u[x]-u[x-1])/h

with eroded/edge1/edge2 derived from a binary domain mask. For the
all-ones mask this reduces to a central difference along x with
one-sided differences at the two edge columns of each row.

Sharding: data-parallel over batch B=8 -> 8 NeuronCores (the stencil is
along the innermost x axis, so no halo is needed). Each core processes
u[b] of shape (4, 1024, 1024) = 16MB, viewed as a flat (2048, 2048)
matrix: each SBUF tile partition holds 2 consecutive image rows side by
side in the free dimension. Per (128, 2048) tile:
  1 DVE subtract over the shifted tile (central difference; the
  1024-boundary seams inside the free dim produce garbage that is
  overwritten),
  1 strided DVE subtract + 1 strided DVE scalar-mul for the 4 edge
  columns (one-sided differences, x2),
  1 ScalarE activation Copy with scale=1/(2h),
then DMA out. ~32MB of HBM traffic per core => DMA-bound at ~300 GB/s
per core sustained (~105 us/core measured by loop-slope timing).
"""

import numpy as np

H_SPACING = 0.01
B, C, HGT, W = 8, 4, 1024, 1024
N_CORES = 8
FREE = 2048              # flat-view row length (2 image rows per partition)
ROWS = C * HGT * W // FREE  # 2048 rows in the flat per-core view
P = 128                  # SBUF partitions
BUFS = (6, 4, 6)         # in / diff / out tile pool depths

_cached_nc = None


def _build_program():
    import concourse.bacc as bacc
    import concourse.mybir as mybir
    import concourse.tile as tile

    f32 = mybir.dt.float32
    Copy = mybir.ActivationFunctionType.Copy
    scale = 1.0 / (2.0 * H_SPACING)
    nb = FREE // W
    bi, bd, bo = BUFS

    nc = bacc.Bacc("TRN2", target_bir_lowering=False, debug=False)
    u = nc.dram_tensor("u", (ROWS, FREE), f32, kind="ExternalInput").ap()
    out = nc.dram_tensor("out", (ROWS, FREE), f32, kind="ExternalOutput").ap()

    with tile.TileContext(nc) as tc:
        with (
            tc.tile_pool(name="tin", bufs=bi) as tin,
            tc.tile_pool(name="tdiff", bufs=bd) as tdiff,
            tc.tile_pool(name="tout", bufs=bo) as tout,
        ):
            for t in range(ROWS // P):
                T = tin.tile([P, FREE], f32)
                nc.sync.dma_start(T[:], u[t * P:(t + 1) * P, :])

                D = tdiff.tile([P, FREE], f32)
                # Central difference everywhere; wrong at the block-edge
                # columns (incl. cross-seam reads), fixed up below.
                nc.vector.tensor_sub(D[:, 1:FREE - 1], T[:, 2:FREE], T[:, 0:FREE - 2])

                T3 = T[:].rearrange("p (b x) -> p b x", b=nb)
                D3 = D[:].rearrange("p (b x) -> p b x", b=nb)
                # Block-relative: D[0] = u[1]-u[0]; D[W-1] = u[W-1]-u[W-2]
                nc.vector.tensor_sub(
                    D3[:, :, 0:W:W - 1],
                    T3[:, :, 1:W:W - 2],
                    T3[:, :, 0:W - 1:W - 2],
                )
                # One-sided difference is /h, not /(2h): pre-double.
                nc.vector.tensor_scalar_mul(
                    D3[:, :, 0:W:W - 1], D3[:, :, 0:W:W - 1], 2.0
                )

                O = tout.tile([P, FREE], f32)
                nc.scalar.activation(O[:], D[:], Copy, scale=scale)
                nc.sync.dma_start(out[t * P:(t + 1) * P, :], O[:])
    nc.compile()
    return nc


def _general_numpy(u, nmask):
    # Fallback for a non-trivial domain mask (never hit for the shipped
    # inputs, where nmask is all ones): the reference formula in numpy.
    h = H_SPACING
    up = np.pad(u, ((0, 0), (0, 0), (0, 0), (1, 1)))
    u_r = up[..., 2:]
    u_l = up[..., :-2]
    internal_d = (u_r - u_l) / (2.0 * h)
    left_d = (u_r - u) / h
    right_d = (u - u_l) / h
    mp = np.pad(nmask, ((0, 0), (0, 0), (0, 0), (1, 1)))
    eroded = ((mp[..., :-2] + nmask + mp[..., 2:]) == 3.0).astype(u.dtype)
    diffs = mp[..., 1:] - mp[..., :-1]
    edge1 = (diffs[..., :-1] == 1.0).astype(u.dtype)
    edge2 = (diffs[..., 1:] == -1.0).astype(u.dtype)
    return eroded * internal_d + edge1 * left_d + edge2 * right_d


def kernel(u, nmask):
    u = np.asarray(u, dtype=np.float32)
    nmask = np.asarray(nmask, dtype=np.float32)
    if not np.all(nmask == 1.0):
        return _general_numpy(u, nmask)

    global _cached_nc
    if _cached_nc is None:
        _cached_nc = _build_program()
    nc = _cached_nc

    from concourse.bass_utils import run_bass_kernel_spmd

    in_maps = [
        {"u": np.ascontiguousarray(u[b].reshape(ROWS, FREE))} for b in range(B)
    ]
    res = run_bass_kernel_spmd(nc, in_maps, list(range(N_CORES)))
    return np.stack(
        [res.results[b]["out"].reshape(C, HGT, W) for b in range(B)]
    ).astype(np.float32)


# revision 6
# speedup vs baseline: 1.0031x; 1.0031x over previous
"""Trainium2 Bass kernel for DerivativeNet.forward(u, direction='x').

out = eroded * (u[x+1]-u[x-1])/(2h) + edge1 * (u[x+1]-u[x])/h + edge2 * (u[x]-u[x-1])/h

with eroded/edge1/edge2 derived from a binary domain mask. For the
all-ones mask this reduces to a central difference along x with
one-sided differences at the two edge columns of each row.

Sharding: data-parallel over batch B=8 -> 8 NeuronCores (the stencil is
along the innermost x axis, so no halo is needed). Each core processes
u[b] of shape (4, 1024, 1024) = 16MB, viewed as a flat (2048, 2048)
matrix: each SBUF tile partition holds 2 consecutive image rows side by
side in the free dimension. Per (128, 2048) tile:
  1 DVE subtract over the shifted tile (central difference; the
  1024-boundary seams inside the free dim produce garbage that is
  overwritten),
  1 strided DVE subtract + 1 strided DVE scalar-mul for the 4 edge
  columns (one-sided differences, x2),
  1 ScalarE activation Copy with scale=1/(2h),
then DMA out. ~32MB of HBM traffic per core => DMA-bound at ~300 GB/s
per core sustained (~105 us/core measured by loop-slope timing).
"""

import numpy as np

H_SPACING = 0.01
B, C, HGT, W = 8, 4, 1024, 1024
N_CORES = 8
FREE = 2048              # flat-view row length (2 image rows per partition)
ROWS = C * HGT * W // FREE  # 2048 rows in the flat per-core view
P = 128                  # SBUF partitions
BUFS = (10, 4, 4)        # in / diff / out pool depths: deep load prefetch
                         # shortens the single-shot ramp; 4 store slots
                         # keep the out-stream pipelined.

_cached_nc = None


def _build_program():
    import concourse.bacc as bacc
    import concourse.mybir as mybir
    import concourse.tile as tile

    f32 = mybir.dt.float32
    Copy = mybir.ActivationFunctionType.Copy
    scale = 1.0 / (2.0 * H_SPACING)
    nb = FREE // W
    bi, bd, bo = BUFS

    nc = bacc.Bacc("TRN2", target_bir_lowering=False, debug=False)
    u = nc.dram_tensor("u", (ROWS, FREE), f32, kind="ExternalInput").ap()
    out = nc.dram_tensor("out", (ROWS, FREE), f32, kind="ExternalOutput").ap()

    with tile.TileContext(nc) as tc:
        with (
            tc.tile_pool(name="tin", bufs=bi) as tin,
            tc.tile_pool(name="tdiff", bufs=bd) as tdiff,
            tc.tile_pool(name="tout", bufs=bo) as tout,
        ):
            for t in range(ROWS // P):
                T = tin.tile([P, FREE], f32)
                nc.sync.dma_start(T[:], u[t * P:(t + 1) * P, :])

                D = tdiff.tile([P, FREE], f32)
                # Central difference everywhere; wrong at the block-edge
                # columns (incl. cross-seam reads), fixed up below.
                nc.vector.tensor_sub(D[:, 1:FREE - 1], T[:, 2:FREE], T[:, 0:FREE - 2])

                T3 = T[:].rearrange("p (b x) -> p b x", b=nb)
                D3 = D[:].rearrange("p (b x) -> p b x", b=nb)
                # Block-relative: D[0] = u[1]-u[0]; D[W-1] = u[W-1]-u[W-2]
                nc.vector.tensor_sub(
                    D3[:, :, 0:W:W - 1],
                    T3[:, :, 1:W:W - 2],
                    T3[:, :, 0:W - 1:W - 2],
                )
                # One-sided difference is /h, not /(2h): pre-double.
                nc.vector.tensor_scalar_mul(
                    D3[:, :, 0:W:W - 1], D3[:, :, 0:W:W - 1], 2.0
                )

                O = tout.tile([P, FREE], f32)
                nc.scalar.activation(O[:], D[:], Copy, scale=scale)
                # Stores go out on the ACT HWDGE ring (qActDynamicHW),
                # loads on the SP ring (qSPDynamicHW): HWDGE DMAs are
                # FIFO-ordered per issuing engine, so separate rings
                # decouple the load and store streams.
                nc.scalar.dma_start(out[t * P:(t + 1) * P, :], O[:])
    nc.compile()
    return nc


def _general_numpy(u, nmask):
    # Fallback for a non-trivial domain mask (never hit for the shipped
    # inputs, where nmask is all ones): the reference formula in numpy.
    h = H_SPACING
    up = np.pad(u, ((0, 0), (0, 0), (0, 0), (1, 1)))
    u_r = up[..., 2:]
    u_l = up[..., :-2]
    internal_d = (u_r - u_l) / (2.0 * h)
    left_d = (u_r - u) / h
    right_d = (u - u_l) / h
    mp = np.pad(nmask, ((0, 0), (0, 0), (0, 0), (1, 1)))
    eroded = ((mp[..., :-2] + nmask + mp[..., 2:]) == 3.0).astype(u.dtype)
    diffs = mp[..., 1:] - mp[..., :-1]
    edge1 = (diffs[..., :-1] == 1.0).astype(u.dtype)
    edge2 = (diffs[..., 1:] == -1.0).astype(u.dtype)
    return eroded * internal_d + edge1 * left_d + edge2 * right_d


def kernel(u, nmask):
    u = np.asarray(u, dtype=np.float32)
    nmask = np.asarray(nmask, dtype=np.float32)
    if not np.all(nmask == 1.0):
        return _general_numpy(u, nmask)

    global _cached_nc
    if _cached_nc is None:
        _cached_nc = _build_program()
    nc = _cached_nc

    from concourse.bass_utils import run_bass_kernel_spmd

    in_maps = [
        {"u": np.ascontiguousarray(u[b].reshape(ROWS, FREE))} for b in range(B)
    ]
    res = run_bass_kernel_spmd(nc, in_maps, list(range(N_CORES)))
    return np.stack(
        [res.results[b]["out"].reshape(C, HGT, W) for b in range(B)]
    ).astype(np.float32)
